# revision 1
# baseline (speedup 1.0000x reference)
"""Trainium2 Bass kernel for DetectionSegmentationConsistency loss.

Per-box sums over seg-mask rectangles are computed as a masked matmul:
  diff  = footpath - driveway                    (DVE, bf16 out)
  T     = R^T.T @ diff  accumulated over 8 row-chunks in PSUM
          where R^T[h, n] = (y1[n] <= h < y2[n]) row-range indicator (bf16)
  S[n]  = sum_x T[n, x] * (x1[n] <= x < x2[n])   (two fused scalar_tensor_tensor)
  loss += relu(S[n]) * conf[n] * valid[n] / area[n]

Data-parallel across 8 NeuronCores: each core takes 4 of the 32 batch images
(only seg classes 1 and 2 are shipped) and emits one partial-sum scalar;
host adds the 8 partials and divides by B*N.

Instruction-dependency hygiene: walrus allows very few semaphore waits per
instruction, so every tile is written by at most one DMA, and cross-engine
fan-in is kept minimal (e.g. iotas are bounced through a DVE copy).
"""
import numpy as np
from contextlib import ExitStack

import concourse.bass as bass
import concourse.bacc as bacc
import concourse.tile as tile
from concourse import mybir
from concourse.bass_utils import run_bass_kernel_spmd

F32 = mybir.dt.float32
BF16 = mybir.dt.bfloat16
I32 = mybir.dt.int32

B, N, H, W = 32, 300, 1024, 1024
NCORES = 8
BC = B // NCORES                # images per core
KCH = H // 128                  # 8 row chunks
NG = [128, 128, 44]             # box groups along partitions
GS = [0, 128, 256]
CONF_THRESH = 0.3
MAGIC = 12582912.0              # 1.5 * 2^23: fp32 round-to-nearest-int trick

AluOp = mybir.AluOpType
Act = mybir.ActivationFunctionType


def _floor_clip(nc, pool, val_ap, out_ap, p, lo, hi):
    """out = clip(floor(val), lo, hi), exact fp32 (magic-number RN + fixup)."""
    fd = val_ap.shape[1]
    r = pool.tile([128, fd], F32, tag="fc_r", name="fc_r")
    gt = pool.tile([128, fd], F32, tag="fc_g", name="fc_g")
    nc.vector.tensor_scalar(
        out=r[:p], in0=val_ap, scalar1=MAGIC, scalar2=MAGIC,
        op0=AluOp.add, op1=AluOp.subtract)
    nc.vector.tensor_tensor(out=gt[:p], in0=r[:p], in1=val_ap, op=AluOp.is_gt)
    nc.vector.tensor_tensor(out=r[:p], in0=r[:p], in1=gt[:p], op=AluOp.subtract)
    nc.vector.tensor_scalar(
        out=out_ap, in0=r[:p], scalar1=float(lo), scalar2=float(hi),
        op0=AluOp.max, op1=AluOp.min)


def build_bass():
    # Bacc (not raw Bass): its finalize() runs move_matmul_waits_to_ldweights
    # + generate_event_semaphores, which legalize multi-sem waits for walrus.
    nc = bacc.Bacc()
    seg = nc.declare_dram_parameter("seg2", [BC, 2, H, W], F32, isOutput=False)
    boxes = nc.declare_dram_parameter("boxes", [BC, N, 4], F32, isOutput=False)
    conf = nc.declare_dram_parameter("conf", [BC, N], F32, isOutput=False)
    out = nc.declare_dram_parameter("out", [1, 1], F32, isOutput=True)

    with tile.TileContext(nc) as tc, ExitStack() as ctx:
        consts = ctx.enter_context(tc.tile_pool(name="consts", bufs=1))
        boxp = ctx.enter_context(tc.tile_pool(name="boxp", bufs=1))
        scratch = ctx.enter_context(tc.tile_pool(name="scratch", bufs=4))
        segp = ctx.enter_context(tc.tile_pool(name="segp", bufs=3))
        diffp = ctx.enter_context(tc.tile_pool(name="diffp", bufs=4))
        maskp = ctx.enter_context(tc.tile_pool(name="maskp", bufs=4))
        bcp = ctx.enter_context(tc.tile_pool(name="bcp", bufs=2))
        cmp_ = ctx.enter_context(tc.tile_pool(name="cmp", bufs=2))
        psum = ctx.enter_context(tc.tile_pool(name="psum", bufs=1, space="PSUM"))
        dramp = ctx.enter_context(tc.tile_pool(name="dramp", bufs=1, space="DRAM"))

        # ---- constants (iotas bounce through DVE so consumers only dep DVE) --
        iotaF_i = consts.tile([128, W], I32)
        nc.gpsimd.iota(iotaF_i, pattern=[[1, W]], base=0, channel_multiplier=0)
        iotaF = consts.tile([128, W], F32)
        nc.vector.tensor_copy(iotaF, iotaF_i)

        iotaP_i = consts.tile([128, KCH], I32)
        nc.gpsimd.iota(iotaP_i, pattern=[[128, KCH]], base=0, channel_multiplier=1)
        iotaP = consts.tile([128, KCH], F32)  # [p, k] = p + 128*k
        nc.vector.tensor_copy(iotaP, iotaP_i)

        ones_col = consts.tile([128, 1], F32)
        nc.vector.memset(ones_col, 1.0)

        # ---- per-box params, column layout (boxes on partitions) ----
        x1c, x2c, wgt, scol = [], [], [], []
        for g in range(3):
            p, s = NG[g], GS[g]
            bx = boxp.tile([128, BC, 4], F32, tag=f"bx{g}")
            nc.sync.dma_start(
                out=bx[:p], in_=boxes[:, s:s + p, :].rearrange("b n c -> n b c"))
            cf = boxp.tile([128, BC], F32, tag=f"cf{g}")
            nc.sync.dma_start(
                out=cf[:p], in_=conf[:, s:s + p].rearrange("b n -> n b"))

            cx, cy = bx[:p, :, 0], bx[:p, :, 1]
            ww, hh = bx[:p, :, 2], bx[:p, :, 3]
            w512 = scratch.tile([128, BC], F32, tag="w512")
            h512 = scratch.tile([128, BC], F32, tag="h512")
            nc.vector.tensor_scalar_mul(w512[:p], ww, 512.0)
            nc.vector.tensor_scalar_mul(h512[:p], hh, 512.0)

            x1g = boxp.tile([128, BC], F32, tag=f"x1c{g}")
            x2g = boxp.tile([128, BC], F32, tag=f"x2c{g}")
            y1g = boxp.tile([128, BC], F32, tag=f"y1c{g}")
            y2g = boxp.tile([128, BC], F32, tag=f"y2c{g}")
            for (vout, base, half, op1) in (
                (x1g, cx, w512, AluOp.subtract),
                (x2g, cx, w512, AluOp.add),
                (y1g, cy, h512, AluOp.subtract),
                (y2g, cy, h512, AluOp.add),
            ):
                vf = scratch.tile([128, BC], F32, tag="vf", name="vf")
                nc.vector.scalar_tensor_tensor(
                    out=vf[:p], in0=base, scalar=1024.0, in1=half[:p],
                    op0=AluOp.mult, op1=op1)
                _floor_clip(nc, scratch, vf[:p], vout[:p], p, 0.0, 1023.0)

            # weight = conf * (conf >= .3) / max(area, 1)
            dx = scratch.tile([128, BC], F32, tag="dx")
            dy = scratch.tile([128, BC], F32, tag="dy")
            nc.vector.tensor_tensor(out=dx[:p], in0=x2g[:p], in1=x1g[:p], op=AluOp.subtract)
            nc.vector.tensor_tensor(out=dy[:p], in0=y2g[:p], in1=y1g[:p], op=AluOp.subtract)
            area = scratch.tile([128, BC], F32, tag="area")
            nc.vector.tensor_tensor(out=area[:p], in0=dx[:p], in1=dy[:p], op=AluOp.mult)
            nc.vector.tensor_scalar_max(area[:p], area[:p], 1.0)
            rsa = scratch.tile([128, BC], F32, tag="rsa")
            nc.vector.reciprocal(rsa[:p], area[:p])
            vmask = scratch.tile([128, BC], F32, tag="vmask")
            nc.vector.tensor_scalar(
                out=vmask[:p], in0=cf[:p], scalar1=CONF_THRESH, scalar2=None,
                op0=AluOp.is_ge)
            wg = boxp.tile([128, BC], F32, tag=f"wgt{g}")
            nc.vector.tensor_tensor(out=wg[:p], in0=cf[:p], in1=vmask[:p], op=AluOp.mult)
            nc.vector.tensor_tensor(out=wg[:p], in0=wg[:p], in1=rsa[:p], op=AluOp.mult)

            sc = boxp.tile([128, BC], F32, tag=f"scol{g}")
            x1c.append(x1g); x2c.append(x2g)
            wgt.append(wg); scol.append(sc)

        # ---- y rows (images on partitions 0..3), all on DVE ----
        boxrow = boxp.tile([BC, N, 4], F32, tag="boxrow")
        nc.sync.dma_start(out=boxrow, in_=boxes[:, :, :])
        cyr, hhr = boxrow[:, :, 1], boxrow[:, :, 3]
        h512r = scratch.tile([BC, N], F32, tag="h512r")
        nc.vector.tensor_scalar_mul(h512r, hhr, 512.0)
        y1row = boxp.tile([BC, N], F32, tag="y1row")
        y2row = boxp.tile([BC, N], F32, tag="y2row")
        for (vout, op1) in ((y1row, AluOp.subtract), (y2row, AluOp.add)):
            vf = scratch.tile([BC, N], F32, tag="vfr", name="vfr")
            nc.vector.scalar_tensor_tensor(
                out=vf, in0=cyr, scalar=1024.0, in1=h512r,
                op0=AluOp.mult, op1=op1)
            _floor_clip(nc, scratch, vf[:BC], vout[:BC], BC, 0.0, 1023.0)

        # broadcast each image's y-row down 128 partitions via a DRAM bounce
        # (SBUF APs cannot have partition step 0, DRAM APs can)
        ybounce = dramp.tile([2, BC, N], F32)
        nc.gpsimd.dma_start(out=ybounce[0], in_=y1row[:, :])
        nc.gpsimd.dma_start(out=ybounce[1], in_=y2row[:, :])
        y1bc, y2bc = [], []
        for b in range(BC):
            for j in range(2):
                bc_sb = bcp.tile([128, N], F32, tag=f"ybc{j}", name=f"ybc{j}_{b}")
                nc.gpsimd.dma_start(
                    out=bc_sb, in_=ybounce[j, b:b + 1, :].to_broadcast((128, N)))
                (y1bc if j == 0 else y2bc).append(bc_sb)

        # ---- main loop over images ----
        for b in range(BC):
            tps = [psum.tile([NG[g], 1024], F32, tag=f"T{g}", name=f"T{g}_{b}")
                   for g in range(3)]
            for k in range(KCH):
                seg_t = segp.tile([128, 2, W], F32, tag="seg")
                nc.gpsimd.dma_start(
                    out=seg_t,
                    in_=seg[b, :, k * 128:(k + 1) * 128, :].rearrange("c p w -> p c w"))
                diff = diffp.tile([128, W], BF16, tag="diff")
                nc.vector.tensor_tensor(
                    out=diff, in0=seg_t[:, 1, :], in1=seg_t[:, 0, :], op=AluOp.subtract)

                m2 = maskp.tile([128, N], F32, tag="m2")
                nc.vector.tensor_scalar(
                    out=m2, in0=y2bc[b], scalar1=iotaP[:, k:k + 1], scalar2=None,
                    op0=AluOp.is_gt)
                rt = maskp.tile([128, N], BF16, tag="rt")
                nc.vector.scalar_tensor_tensor(
                    out=rt, in0=y1bc[b], scalar=iotaP[:, k:k + 1], in1=m2,
                    op0=AluOp.is_le, op1=AluOp.mult)

                for g in range(3):
                    p, s = NG[g], GS[g]
                    for half in range(2):
                        nc.tensor.matmul(
                            out=tps[g][:, half * 512:(half + 1) * 512],
                            lhsT=rt[:, s:s + p],
                            rhs=diff[:, half * 512:(half + 1) * 512],
                            start=(k == 0), stop=(k == KCH - 1))

            for g in range(3):
                p = NG[g]
                masked = cmp_.tile([NG[g], 1024], F32, tag="masked", name=f"masked{g}_{b}")
                nc.vector.scalar_tensor_tensor(
                    out=masked, in0=iotaF[:p], scalar=x1c[g][:p, b:b + 1],
                    in1=tps[g], op0=AluOp.is_ge, op1=AluOp.mult)
                junk = cmp_.tile([NG[g], 1024], F32, tag="junk", name=f"junk{g}_{b}")
                nc.vector.scalar_tensor_tensor(
                    out=junk, in0=iotaF[:p], scalar=x2c[g][:p, b:b + 1],
                    in1=masked, op0=AluOp.is_lt, op1=AluOp.mult,
                    accum_out=scol[g][:p, b:b + 1])

        # ---- final: relu(S)*wgt, reduce boxes+images, partition-reduce ----
        fin = psum.tile([1, 1], F32, tag="fin")
        for g in range(3):
            p = NG[g]
            pb = scratch.tile([128, BC], F32, tag="pb")
            nc.vector.tensor_tensor(out=pb[:p], in0=scol[g][:p], in1=wgt[g][:p], op=AluOp.mult)
            rl = scratch.tile([128, BC], F32, tag="rl")
            nc.vector.tensor_relu(rl[:p], pb[:p])
            rs = scratch.tile([128, 1], F32, tag="rs")
            nc.vector.reduce_sum(out=rs[:p], in_=rl[:p], axis=mybir.AxisListType.X)
            nc.tensor.matmul(
                out=fin, lhsT=ones_col[:p], rhs=rs[:p],
                start=(g == 0), stop=(g == 2))
        fsb = scratch.tile([1, 1], F32, tag="fsb")
        nc.scalar.copy(out=fsb, in_=fin)
        nc.sync.dma_start(out=out[0:1, 0:1], in_=fsb)

    nc.finalize()
    return nc


_NC_CACHE = None


def _get_nc():
    global _NC_CACHE
    if _NC_CACHE is None:
        _NC_CACHE = build_bass()
    return _NC_CACHE


def kernel(det_boxes, det_confidence, seg_masks):
    det_boxes = np.ascontiguousarray(np.asarray(det_boxes, dtype=np.float32))
    det_confidence = np.ascontiguousarray(np.asarray(det_confidence, dtype=np.float32))
    seg_masks = np.asarray(seg_masks, dtype=np.float32)

    nc = _get_nc()
    in_maps = []
    for i in range(NCORES):
        sl = slice(BC * i, BC * (i + 1))
        in_maps.append({
            "seg2": np.ascontiguousarray(seg_masks[sl, 1:3]),
            "boxes": det_boxes[sl],
            "conf": det_confidence[sl],
        })
    res = run_bass_kernel_spmd(nc, in_maps, list(range(NCORES)))
    parts = np.array([res.results[i]["out"][0, 0] for i in range(NCORES)],
                     dtype=np.float32)
    total = np.sum(parts, dtype=np.float32) / np.float32(B * N)
    return np.array(total, dtype=np.float32)



# revision 9
# speedup vs baseline: 1.7379x; 1.7379x over previous
"""Trainium2 Bass kernel for DetectionSegmentationConsistency loss.

Per-box rectangle sums over (footpath - driveway) are computed as masked
matmuls in fp8 with DoubleRow perf mode (0.5 PE cycles/row):

  host ships  S[b, sc, p, pl, j, x] = plane_pl at (y = 256*sc + 128*j + p, x)
              with plane 0 = footpath, plane 1 = -driveway, quantized e4m3
  y-masks     M[p, j, n] = (y1[n] <= Y < y2[n]),  Y = 256*sc + 128*j + p
              s1 = sigmoid(64*(Y - y1) + 32) on the Act engine (exact {0,1})
              M  = (y2 > Y) * s1 fused on DVE (fp8 out)
  T[n, x]     = sum_sc sum_pl  M^T @ S[pl]   PSUM f32, DoubleRow contraction
              over 256 y-rows per matmul (both planes share the same mask)
  S[n]        = sum_x T[n, x] * (x1 <= x < x2): Act copies T to f16 SBUF,
              then two fused scalar_tensor_tensor ops in all-f16 (2x DVE mode)
  loss       += relu(S) * conf * valid / area

Data-parallel across 8 NeuronCores: 4 images per core, one partial sum each;
host adds partials and divides by B*N.
"""
import numpy as np
from contextlib import ExitStack

import concourse.bass as bass
import concourse.bacc as bacc
import concourse.tile as tile
from concourse import mybir
from concourse.bass_utils import run_bass_kernel_spmd

F32 = mybir.dt.float32
F16 = mybir.dt.float16
F8 = mybir.dt.float8e4
I32 = mybir.dt.int32
F8NP = mybir.dt.np(mybir.dt.float8e4)

B, N, H, W = 32, 300, 1024, 1024
NCORES = 8
BC = B // NCORES                # images per core
SC = 4                          # super-chunks of 256 y-rows
NG = [128, 128, 44]             # box groups along partitions
GS = [0, 128, 256]
CONF_THRESH = 0.3
MAGIC = 12582912.0              # 1.5 * 2^23: fp32 round-to-nearest-int trick

AluOp = mybir.AluOpType
Act = mybir.ActivationFunctionType
DR = mybir.MatmulPerfMode.DoubleRow


def _floor_clip(nc, pool, val_ap, out_ap, p, lo, hi):
    """out = clip(floor(val), lo, hi), exact fp32 (magic-number RN + fixup)."""
    fd = val_ap.shape[1]
    r = pool.tile([128, fd], F32, tag="fc_r", name="fc_r")
    gt = pool.tile([128, fd], F32, tag="fc_g", name="fc_g")
    nc.vector.tensor_scalar(
        out=r[:p], in0=val_ap, scalar1=MAGIC, scalar2=MAGIC,
        op0=AluOp.add, op1=AluOp.subtract)
    nc.vector.tensor_tensor(out=gt[:p], in0=r[:p], in1=val_ap, op=AluOp.is_gt)
    nc.vector.tensor_tensor(out=r[:p], in0=r[:p], in1=gt[:p], op=AluOp.subtract)
    nc.vector.tensor_scalar(
        out=out_ap, in0=r[:p], scalar1=float(lo), scalar2=float(hi),
        op0=AluOp.max, op1=AluOp.min)


def build_bass():
    nc = bacc.Bacc()
    seg = nc.declare_dram_parameter("seg8", [BC, SC, 128, 2, 2, 2, 512], F8, isOutput=False)
    boxes = nc.declare_dram_parameter("boxes", [BC, N, 4], F32, isOutput=False)
    conf = nc.declare_dram_parameter("conf", [BC, N], F32, isOutput=False)
    out = nc.declare_dram_parameter("out", [1, 1], F32, isOutput=True)

    with tile.TileContext(nc) as tc, ExitStack() as ctx:
        consts = ctx.enter_context(tc.tile_pool(name="consts", bufs=1))
        boxp = ctx.enter_context(tc.tile_pool(name="boxp", bufs=1))
        scratch = ctx.enter_context(tc.tile_pool(name="scratch", bufs=4))
        segp = ctx.enter_context(tc.tile_pool(name="segp", bufs=4))
        maskp = ctx.enter_context(tc.tile_pool(name="maskp", bufs=3))
        s1p = ctx.enter_context(tc.tile_pool(name="s1p", bufs=3))
        tfp = ctx.enter_context(tc.tile_pool(name="tfp", bufs=3))
        xsp = ctx.enter_context(tc.tile_pool(name="xsp", bufs=4))
        bcp = ctx.enter_context(tc.tile_pool(name="bcp", bufs=4))
        psum = ctx.enter_context(tc.tile_pool(name="psum", bufs=1, space="PSUM"))
        dramp = ctx.enter_context(tc.tile_pool(name="dramp", bufs=1, space="DRAM"))

        # ---- constants (iotas bounce through DVE so consumers only dep DVE) --
        iotaF_i = consts.tile([128, W], I32)
        nc.gpsimd.iota(iotaF_i, pattern=[[1, W]], base=0, channel_multiplier=0)
        iotaF16 = consts.tile([128, W], F16)
        nc.vector.tensor_copy(iotaF16, iotaF_i)

        iotaP_i = consts.tile([128, 8], I32)
        nc.gpsimd.iota(iotaP_i, pattern=[[128, 8]], base=0, channel_multiplier=1)
        iotaP = consts.tile([128, 8], F32)  # [p, k] = p + 128*k
        nc.vector.tensor_copy(iotaP, iotaP_i)
        bias64 = consts.tile([128, 8], F32)  # 64*Y + 32
        nc.vector.tensor_scalar(
            out=bias64, in0=iotaP, scalar1=64.0, scalar2=32.0,
            op0=AluOp.mult, op1=AluOp.add)

        ones_col = consts.tile([128, 1], F32)
        nc.vector.memset(ones_col, 1.0)

        # ---- per-box params, column layout (boxes on partitions) ----
        x1c, x2c, wgt, scol = [], [], [], []
        for g in range(3):
            p, s = NG[g], GS[g]
            bx = boxp.tile([128, BC, 4], F32, tag=f"bx{g}")
            nc.sync.dma_start(
                out=bx[:p], in_=boxes[:, s:s + p, :].rearrange("b n c -> n b c"))
            cf = boxp.tile([128, BC], F32, tag=f"cf{g}")
            nc.sync.dma_start(
                out=cf[:p], in_=conf[:, s:s + p].rearrange("b n -> n b"))

            cx, cy = bx[:p, :, 0], bx[:p, :, 1]
            ww, hh = bx[:p, :, 2], bx[:p, :, 3]
            w512 = scratch.tile([128, BC], F32, tag="w512")
            h512 = scratch.tile([128, BC], F32, tag="h512")
            nc.vector.tensor_scalar_mul(w512[:p], ww, 512.0)
            nc.vector.tensor_scalar_mul(h512[:p], hh, 512.0)

            x1g = boxp.tile([128, BC], F32, tag=f"x1c{g}")
            x2g = boxp.tile([128, BC], F32, tag=f"x2c{g}")
            y1g = boxp.tile([128, BC], F32, tag=f"y1c{g}")
            y2g = boxp.tile([128, BC], F32, tag=f"y2c{g}")
            for (vout, base, half, op1) in (
                (x1g, cx, w512, AluOp.subtract),
                (x2g, cx, w512, AluOp.add),
                (y1g, cy, h512, AluOp.subtract),
                (y2g, cy, h512, AluOp.add),
            ):
                vf = scratch.tile([128, BC], F32, tag="vf", name="vf")
                nc.vector.scalar_tensor_tensor(
                    out=vf[:p], in0=base, scalar=1024.0, in1=half[:p],
                    op0=AluOp.mult, op1=op1)
                _floor_clip(nc, scratch, vf[:p], vout[:p], p, 0.0, 1023.0)

            # weight = conf * (conf >= .3) / max(area, 1)
            dx = scratch.tile([128, BC], F32, tag="dx")
            dy = scratch.tile([128, BC], F32, tag="dy")
            nc.vector.tensor_tensor(out=dx[:p], in0=x2g[:p], in1=x1g[:p], op=AluOp.subtract)
            nc.vector.tensor_tensor(out=dy[:p], in0=y2g[:p], in1=y1g[:p], op=AluOp.subtract)
            area = scratch.tile([128, BC], F32, tag="area")
            nc.vector.tensor_tensor(out=area[:p], in0=dx[:p], in1=dy[:p], op=AluOp.mult)
            nc.vector.tensor_scalar_max(area[:p], area[:p], 1.0)
            rsa = scratch.tile([128, BC], F32, tag="rsa")
            nc.vector.reciprocal(rsa[:p], area[:p])
            vmask = scratch.tile([128, BC], F32, tag="vmask")
            nc.vector.tensor_scalar(
                out=vmask[:p], in0=cf[:p], scalar1=CONF_THRESH, scalar2=None,
                op0=AluOp.is_ge)
            wg = boxp.tile([128, BC], F32, tag=f"wgt{g}")
            nc.vector.tensor_tensor(out=wg[:p], in0=cf[:p], in1=vmask[:p], op=AluOp.mult)
            nc.vector.tensor_tensor(out=wg[:p], in0=wg[:p], in1=rsa[:p], op=AluOp.mult)

            sc_ = boxp.tile([128, BC], F32, tag=f"scol{g}")
            x1c.append(x1g); x2c.append(x2g)
            wgt.append(wg); scol.append(sc_)

        # ---- y rows (images on partitions 0..3), all on DVE ----
        boxrow = boxp.tile([BC, N, 4], F32, tag="boxrow")
        nc.sync.dma_start(out=boxrow, in_=boxes[:, :, :])
        cyr, hhr = boxrow[:, :, 1], boxrow[:, :, 3]
        h512r = scratch.tile([BC, N], F32, tag="h512r")
        nc.vector.tensor_scalar_mul(h512r, hhr, 512.0)
        y1row = boxp.tile([BC, N], F32, tag="y1row")
        y2row = boxp.tile([BC, N], F32, tag="y2row")
        for (vout, op1) in ((y1row, AluOp.subtract), (y2row, AluOp.add)):
            vf = scratch.tile([BC, N], F32, tag="vfr", name="vfr")
            nc.vector.scalar_tensor_tensor(
                out=vf, in0=cyr, scalar=1024.0, in1=h512r,
                op0=AluOp.mult, op1=op1)
            _floor_clip(nc, scratch, vf[:BC], vout[:BC], BC, 0.0, 1023.0)
        # padded to 384 = 3 groups x 128; pad boxes get y1=2000/y2=0 -> mask 0
        y1row16 = boxp.tile([BC, 384], F16, tag="y1row16")
        y2row16 = boxp.tile([BC, 384], F16, tag="y2row16")
        nc.vector.memset(y1row16, 2000.0)
        nc.vector.memset(y2row16, 0.0)
        nc.vector.tensor_copy(y1row16[:, 0:N], y1row)
        nc.vector.tensor_copy(y2row16[:, 0:N], y2row)

        # broadcast each image's y-row down 128 partitions via a DRAM bounce
        # (SBUF APs cannot have partition step 0, DRAM APs can)
        ybounce = dramp.tile([2, BC, 384], F16)
        nc.gpsimd.dma_start(out=ybounce[0], in_=y1row16[:, :])
        nc.gpsimd.dma_start(out=ybounce[1], in_=y2row16[:, :])
        y1bc, y2bc = [], []
        for b in range(BC):
            for j in range(2):
                bc_sb = bcp.tile([128, 3, 128], F16, tag=f"ybc{j}", name=f"ybc{j}_{b}")
                nc.gpsimd.dma_start(
                    out=bc_sb,
                    in_=ybounce[j, b:b + 1, :].rearrange(
                        "o (g n) -> o g n", g=3).to_broadcast((128, 3, 128)))
                (y1bc if j == 0 else y2bc).append(bc_sb)

        # ---- main loop over images ----
        for b in range(BC):
            tps = [psum.tile([128, 1024], F32, tag=f"T{g}", name=f"T{g}_{b}")
                   for g in range(3)]
            for k in range(SC):
                seg_t = segp.tile([128, 2, 2, 2, 512], F8, tag="seg")
                nc.sync.dma_start(out=seg_t, in_=seg[b, k])

                s1 = s1p.tile([128, 2, 3, 128], F8, tag="s1")
                m = maskp.tile([128, 3, 2, 128], F8, tag="m")
                for j in range(2):
                    c = 2 * k + j
                    nc.scalar.activation(
                        out=s1[:, j, :, :], in_=y1bc[b], func=Act.Sigmoid,
                        scale=-64.0, bias=bias64[:, c:c + 1])
                    nc.vector.scalar_tensor_tensor(
                        out=m[:, :, j, :], in0=y2bc[b], scalar=iotaP[:, c:c + 1],
                        in1=s1[:, j, :, :], op0=AluOp.is_gt, op1=AluOp.mult)

                for g in range(3):
                    for pl in range(2):
                        for half in range(2):
                            nc.tensor.matmul(
                                out=tps[g][:, half * 512:(half + 1) * 512],
                                lhsT=m[:, g, :, :],
                                rhs=seg_t[:, pl, half, :, :],
                                start=(k == 0 and pl == 0),
                                stop=(k == SC - 1 and pl == 1),
                                perf_mode=DR)

            for g in range(3):
                p = NG[g]
                t16 = tfp.tile([128, 1024], F16, tag="t16", name=f"t16_{g}_{b}")
                nc.scalar.activation(out=t16[:p], in_=tps[g][:p], func=Act.Copy)
                masked = xsp.tile([128, 1024], F16, tag="masked", name=f"mk{g}_{b}")
                nc.vector.scalar_tensor_tensor(
                    out=masked[:p], in0=iotaF16[:p], scalar=x1c[g][:p, b:b + 1],
                    in1=t16[:p], op0=AluOp.is_ge, op1=AluOp.mult)
                junk = xsp.tile([128, 1024], F16, tag="junk", name=f"jk{g}_{b}")
                nc.vector.scalar_tensor_tensor(
                    out=junk[:p], in0=iotaF16[:p], scalar=x2c[g][:p, b:b + 1],
                    in1=masked[:p], op0=AluOp.is_lt, op1=AluOp.mult,
                    accum_out=scol[g][:p, b:b + 1])

        # ---- final: relu(S)*wgt, reduce boxes+images, partition-reduce ----
        fin = psum.tile([1, 1], F32, tag="fin")
        for g in range(3):
            p = NG[g]
            pb = scratch.tile([128, BC], F32, tag="pb")
            nc.vector.tensor_tensor(out=pb[:p], in0=scol[g][:p], in1=wgt[g][:p], op=AluOp.mult)
            rl = scratch.tile([128, BC], F32, tag="rl")
            nc.vector.tensor_relu(rl[:p], pb[:p])
            rs = scratch.tile([128, 1], F32, tag="rs")
            nc.vector.reduce_sum(out=rs[:p], in_=rl[:p], axis=mybir.AxisListType.X)
            nc.tensor.matmul(
                out=fin, lhsT=ones_col[:p], rhs=rs[:p],
                start=(g == 0), stop=(g == 2))
        fsb = scratch.tile([1, 1], F32, tag="fsb")
        nc.scalar.copy(out=fsb, in_=fin)
        nc.sync.dma_start(out=out[0:1, 0:1], in_=fsb)

    nc.finalize()
    return nc


def prep_core_inputs(det_boxes, det_confidence, seg_masks):
    """Quantize seg planes to fp8 (footpath, -driveway) in the DoubleRow
    super-chunk layout and shard everything across the 8 cores."""
    det_boxes = np.ascontiguousarray(np.asarray(det_boxes, dtype=np.float32))
    det_confidence = np.ascontiguousarray(np.asarray(det_confidence, dtype=np.float32))
    seg_masks = np.asarray(seg_masks, dtype=np.float32)

    f8 = seg_masks[:, 2].astype(F8NP)                      # footpath
    d8 = seg_masks[:, 1].astype(F8NP)                      # driveway
    dn8 = (d8.view(np.uint8) ^ 0x80).view(F8NP)            # -driveway (flip sign)
    # [B, 2pl, H, W] -> [B, SC, 128p, 2pl, 2h, 2j, 512]; y = 256*sc + 128*j + p
    both = np.stack([f8, dn8], axis=1)
    both = both.reshape(B, 2, SC, 2, 128, 2, 512).transpose(0, 2, 4, 1, 5, 3, 6)

    in_maps = []
    for i in range(NCORES):
        sl = slice(BC * i, BC * (i + 1))
        in_maps.append({
            "seg8": np.ascontiguousarray(both[sl]),
            "boxes": det_boxes[sl],
            "conf": det_confidence[sl],
        })
    return in_maps


_NC_CACHE = None


def _get_nc():
    global _NC_CACHE
    if _NC_CACHE is None:
        _NC_CACHE = build_bass()
    return _NC_CACHE


def kernel(det_boxes, det_confidence, seg_masks):
    nc = _get_nc()
    in_maps = prep_core_inputs(det_boxes, det_confidence, seg_masks)
    res = run_bass_kernel_spmd(nc, in_maps, list(range(NCORES)))
    parts = np.array([res.results[i]["out"][0, 0] for i in range(NCORES)],
                     dtype=np.float32)
    total = np.sum(parts, dtype=np.float32) / np.float32(B * N)
    return np.array(total, dtype=np.float32)
